# revision 13
# baseline (speedup 1.0000x reference)
"""Trainium2 Bass kernel for nn_FISA (per-stock TimeLSTM + day LSTM + head).

Sharding: 16 stocks -> 8 cores, 2 stocks/core (stock-parallel, per the hint).
Everything below is hardcoded for S=16, D=64, T=64, E=768, H=64.

Per-core dataflow:
  1. Stream x^T (E x (T-major D)) tiles from DRAM, matmul against Uall_w^T
     k-tiles into PSUM, evict into a resident SBUF Ux buffer laid out
     step-major so the recurrent scan can start after the first chunk.
  2. 64-step TimeLSTM scan, both stocks column-packed (cols = s*64+d):
       cs1-MM (K=65 bias-augmented) -> tanh -> (gpsimd) *(t-1), +c
       gates: identity-MM accumulates Ux_t, 4 per-stock matmuls add Wall@h
       sigmoid -> f*c_adj / i*ct products -> stacked-identity MM adds the
       two 64-row halves -> c'; tanh(c') * o -> h'.
       o-gates copied per step into an OS buffer; a per-step ones-matmul
       column-sums them into day_vecs (PSUM, accumulated across the scan).
  3. Epilogue: transpose day_vecs via PE, day-LSTM (seq_len=1, f-gate drops
     out), hd column-sum, dot with ls_w, +ls_b, leaky relu, DMA out (1,2).

The text/day attention blocks of the reference are dead code (softmax over a
size-1 axis == 1.0), so day_vecs is just sum_h(o-gate) and ft is sum_h(hd).
"""

import sys

sys.path.insert(0, "/opt/trn_rl_repo")

from contextlib import ExitStack

import numpy as np

import concourse.bass as bass
import concourse.tile as tile
from concourse import mybir
from concourse.bass_utils import run_bass_kernel_spmd

F32 = mybir.dt.float32
AF = mybir.ActivationFunctionType

S, D, T, E, H = 16, 64, 64, 768, 64
S_LOC = 2              # stocks per core
N_CORES = 8
DT = T * D             # 4096 columns of x^T per stock, t-major (col = t*64+d)
KT = E // 128          # 6 k-tiles
CHUNK = 512            # x^T columns per matmul chunk (= 8 steps)
NCHUNK = DT // CHUNK   # 8
STEPS_PER_CHUNK = CHUNK // D  # 8


def build_program(split_multiwaits=True):
    nc = bass.Bass(trn_type="TRN2", name="fisa")

    # ---- DRAM I/O (per core) ----
    xt_d = nc.dram_tensor("xt", [S_LOC, KT, 128, DT], F32, kind="ExternalInput")
    uw_d = nc.dram_tensor("uw", [S_LOC, 2, KT, 128, 128], F32, kind="ExternalInput")
    wall_d = nc.dram_tensor("wallT", [S_LOC, 2, 128, 128], F32, kind="ExternalInput")
    wdb_d = nc.dram_tensor("wdb", [S_LOC, 65, 64], F32, kind="ExternalInput")
    tm1_d = nc.dram_tensor("tm1", [64, 64 * 128], F32, kind="ExternalInput")
    ident_d = nc.dram_tensor("ident", [128, 128], F32, kind="ExternalInput")
    vi_d = nc.dram_tensor("vi", [128, 64], F32, kind="ExternalInput")
    lwihg_d = nc.dram_tensor("lwihg", [S_LOC, 3, 65, 64], F32, kind="ExternalInput")
    lswm_d = nc.dram_tensor("lswm", [128, 2], F32, kind="ExternalInput")
    lsb_d = nc.dram_tensor("lsb", [1, 2], F32, kind="ExternalInput")
    out_d = nc.dram_tensor("out", [1, 2], F32, kind="ExternalOutput")

    with tile.TileContext(nc) as tc, ExitStack() as ctx:
        consts = ctx.enter_context(tc.tile_pool(name="consts", bufs=1))
        xpool = ctx.enter_context(tc.tile_pool(name="xtiles", bufs=12))
        ps_ux = ctx.enter_context(tc.tile_pool(name="ps_ux", bufs=2, space="PSUM"))
        ps_cs1 = ctx.enter_context(tc.tile_pool(name="ps_cs1", bufs=2, space="PSUM"))
        ps_g = ctx.enter_context(tc.tile_pool(name="ps_g", bufs=2, space="PSUM"))
        ps_c = ctx.enter_context(tc.tile_pool(name="ps_c", bufs=1, space="PSUM"))
        ps_dv = ctx.enter_context(tc.tile_pool(name="ps_dv", bufs=1, space="PSUM"))
        scr = ctx.enter_context(tc.tile_pool(name="scratch", bufs=3))

        # ---- resident SBUF ----
        uw_sb = consts.tile([128, S_LOC * 2 * KT * 128], F32)   # lhsT k-tiles
        wall_sb = consts.tile([128, S_LOC * 2 * 128], F32)
        wdb_sb = consts.tile([65, S_LOC * 64], F32)
        tm1_sb = consts.tile([64, T * 128], F32)
        ident_sb = consts.tile([128, 128], F32)
        vi_sb = consts.tile([128, 64], F32)
        lwihg_sb = consts.tile([65, S_LOC * 3 * 64], F32)
        lswm_sb = consts.tile([128, 2], F32)
        lsb_sb = consts.tile([1, 2], F32)
        ones_sb = consts.tile([64, 1], F32)
        c_sb = consts.tile([65, 128], F32)      # rows 0:64 c-state, row 64 ones
        h_sb = consts.tile([128, 128], F32)     # rows 0:64 h, row 64 ones, rest 0
        # Ux, step-major: chunk tiles of (128, 8 steps * 256); step block layout
        # [m0: s0 d | s1 d | m1: s0 d | s1 d]
        ux_sb = [consts.tile([128, STEPS_PER_CHUNK * 256], F32, name=f"ux{c}")
                 for c in range(NCHUNK)]
        # o-gate store: 8 tiles of 8 steps each, cols t_local*128 + s*64 + d
        os_sb = [consts.tile([64, STEPS_PER_CHUNK * 128], F32, name=f"os{c}")
                 for c in range(NCHUNK)]

        # ---- const loads ----
        for s in range(S_LOC):
            for m in range(2):
                for k in range(KT):
                    idx = (s * 2 + m) * KT + k
                    nc.sync.dma_start(out=uw_sb[:, idx * 128:(idx + 1) * 128],
                                      in_=uw_d[s, m, k])
                nc.sync.dma_start(
                    out=wall_sb[:, (s * 2 + m) * 128:(s * 2 + m + 1) * 128],
                    in_=wall_d[s, m])
            nc.sync.dma_start(out=wdb_sb[:, s * 64:(s + 1) * 64], in_=wdb_d[s])
            for gi in range(3):
                j = (s * 3 + gi) * 64
                nc.sync.dma_start(out=lwihg_sb[:, j:j + 64], in_=lwihg_d[s, gi])
        nc.sync.dma_start(out=tm1_sb, in_=tm1_d[:, :])
        nc.sync.dma_start(out=ident_sb, in_=ident_d[:, :])
        nc.sync.dma_start(out=vi_sb, in_=vi_d[:, :])
        nc.sync.dma_start(out=lswm_sb, in_=lswm_d[:, :])
        nc.sync.dma_start(out=lsb_sb, in_=lsb_d[:, :])
        nc.vector.memset(ones_sb, 1.0)
        nc.vector.memset(c_sb[0:64, :], 0.0)
        nc.vector.memset(c_sb[64:65, :], 1.0)
        nc.vector.memset(h_sb, 0.0)
        nc.vector.memset(h_sb[64:65, :], 1.0)

        ps_dv_t = ps_dv.tile([128, 64], F32)  # day_vecs ((s,d), t), lives all scan

        evict_flip = [0]

        def emit_ux_chunk(c):
            """Ux matmuls + eviction for chunk c (steps 8c .. 8c+7)."""
            for s in range(S_LOC):
                xts = []
                for k in range(KT):
                    xt_t = xpool.tile([128, CHUNK], F32, tag="x")
                    nc.sync.dma_start(
                        out=xt_t, in_=xt_d[s, k, :, c * CHUNK:(c + 1) * CHUNK])
                    xts.append(xt_t)
                for m in range(2):
                    ps = ps_ux.tile([128, CHUNK], F32, tag="ux")
                    for k in range(KT):
                        idx = (s * 2 + m) * KT + k
                        nc.tensor.matmul(
                            ps, uw_sb[:, idx * 128:(idx + 1) * 128], xts[k],
                            start=(k == 0), stop=(k == KT - 1))
                    # evict into step-major layout: (128, 8, 64) strided view
                    dst = ux_sb[c][:, :].rearrange(
                        "p (t x) -> p t x", x=256)[:, :, m * 128 + s * 64:
                                                   m * 128 + s * 64 + 64]
                    src = ps[:, :].rearrange("p (t d) -> p t d", d=64)
                    if evict_flip[0] % 2 == 0:
                        nc.vector.tensor_copy(out=dst, in_=src)
                    else:
                        nc.scalar.copy(out=dst, in_=src)
                    evict_flip[0] += 1

        def emit_step(t):
            c, tl = t // STEPS_PER_CHUNK, t % STEPS_PER_CHUNK
            # c_s1 = tanh(Wd@c + bd)  (bias via K=65 augmentation)
            ps1 = ps_cs1.tile([64, 128], F32, tag="cs1")
            for s in range(S_LOC):
                nc.tensor.matmul(ps1[:, s * 64:(s + 1) * 64],
                                 wdb_sb[:, s * 64:(s + 1) * 64],
                                 c_sb[:, s * 64:(s + 1) * 64],
                                 start=(s == 0), stop=(s == S_LOC - 1))
            cs1 = scr.tile([64, 128], F32, tag="cs1s")
            nc.scalar.activation(cs1, ps1, AF.Tanh)
            # c_adj = c + cs1*(tvec-1)   (gpsimd, SBUF only)
            ca = scr.tile([64, 128], F32, tag="ca")
            nc.gpsimd.tensor_mul(ca, cs1, tm1_sb[:, t * 128:(t + 1) * 128])
            cadj = scr.tile([64, 128], F32, tag="cadj")
            nc.gpsimd.tensor_add(cadj, ca, c_sb[0:64, :])
            # gates = sigmoid(Ux_t + Wall@h + ub)
            psg = ps_g.tile([128, 256], F32, tag="g")
            nc.tensor.matmul(psg, ident_sb,
                             ux_sb[c][:, tl * 256:(tl + 1) * 256],
                             start=True, stop=False)
            for m in range(2):
                for s in range(S_LOC):
                    last = (m == 1 and s == S_LOC - 1)
                    nc.tensor.matmul(
                        psg[:, m * 128 + s * 64: m * 128 + (s + 1) * 64],
                        wall_sb[:, (s * 2 + m) * 128:(s * 2 + m + 1) * 128],
                        h_sb[:, s * 64:(s + 1) * 64],
                        start=False, stop=last, skip_group_check=not last)
            sg = scr.tile([128, 256], F32, tag="sg")
            nc.scalar.activation(sg, psg, AF.Sigmoid)
            # store o-gates for day_vecs
            nc.gpsimd.tensor_copy(out=os_sb[c][:, tl * 128:(tl + 1) * 128],
                                  in_=sg[0:64, 128:256])
            # c' = f*c_adj + i*ct  via stacked products + VI matmul
            fcic = scr.tile([128, 128], F32, tag="fcic")
            nc.vector.tensor_mul(fcic[0:64, :], sg[0:64, 0:128], cadj)
            nc.vector.tensor_mul(fcic[64:128, :], sg[64:128, 0:128],
                                 sg[64:128, 128:256])
            psc = ps_c.tile([64, 128], F32, tag="c")
            nc.tensor.matmul(psc, vi_sb, fcic, start=True, stop=True)
            nc.scalar.copy(out=c_sb[0:64, :], in_=psc)
            thc = scr.tile([64, 128], F32, tag="thc")
            nc.scalar.activation(thc, psc, AF.Tanh)
            # h' = o * tanh(c')
            nc.vector.tensor_mul(h_sb[0:64, :], sg[0:64, 128:256], thc)
            # day_vecs[:, t] = colsum(o-gates)
            nc.tensor.matmul(ps_dv_t[:, t:t + 1],
                             os_sb[c][:, tl * 128:(tl + 1) * 128],
                             ones_sb, start=(t == 0), stop=(t == T - 1),
                             skip_group_check=(t not in (0, T - 1)))

        # software pipeline: chunk 0 & 1 of Ux first, then interleave
        emit_ux_chunk(0)
        emit_ux_chunk(1)
        for c in range(2, NCHUNK):
            for t in range((c - 2) * STEPS_PER_CHUNK, (c - 1) * STEPS_PER_CHUNK):
                emit_step(t)
            emit_ux_chunk(c)
        for t in range((NCHUNK - 2) * STEPS_PER_CHUNK, T):
            emit_step(t)

        # ---- epilogue ----
        dv_sb = scr.tile([128, 64], F32, tag="dv")
        nc.scalar.copy(out=dv_sb, in_=ps_dv_t)
        ps_dvT = ps_ux.tile([64, 128], F32, tag="ux")
        nc.tensor.transpose(ps_dvT, dv_sb, ident_sb)
        dvt_sb = consts.tile([65, 128], F32)
        nc.vector.memset(dvt_sb[64:65, :], 1.0)
        nc.scalar.copy(out=dvt_sb[0:64, :], in_=ps_dvT)
        # day LSTM gates (i, g, o; f-gate multiplies c0=0 and drops out)
        psg2 = ps_ux.tile([64, 384], F32, tag="ux")
        for gi in range(3):
            for s in range(S_LOC):
                first = (gi == 0 and s == 0)
                last = (gi == 2 and s == S_LOC - 1)
                nc.tensor.matmul(
                    psg2[:, gi * 128 + s * 64: gi * 128 + (s + 1) * 64],
                    lwihg_sb[:, (s * 3 + gi) * 64:(s * 3 + gi + 1) * 64],
                    dvt_sb[:, s * 64:(s + 1) * 64],
                    start=first, stop=last, skip_group_check=not (first or last))
        sg2i = scr.tile([64, 128], F32, tag="e1")
        thg2 = scr.tile([64, 128], F32, tag="e2")
        sg2o = scr.tile([64, 128], F32, tag="e3")
        nc.scalar.activation(sg2i, psg2[:, 0:128], AF.Sigmoid)
        nc.scalar.activation(thg2, psg2[:, 128:256], AF.Tanh)
        nc.scalar.activation(sg2o, psg2[:, 256:384], AF.Sigmoid)
        cd = scr.tile([64, 128], F32, tag="e4")
        nc.vector.tensor_mul(cd, sg2i, thg2)
        thcd = scr.tile([64, 128], F32, tag="e5")
        nc.scalar.activation(thcd, cd, AF.Tanh)
        hd = scr.tile([64, 128], F32, tag="e6")
        nc.vector.tensor_mul(hd, sg2o, thcd)
        # ft = colsum(hd); out = leakyrelu(ft . ls_w + ls_b)
        ps_ft = ps_ux.tile([128, 1], F32, tag="ux")
        nc.tensor.matmul(ps_ft, hd, ones_sb, start=True, stop=True)
        ft_sb = scr.tile([128, 1], F32, tag="e7")
        nc.scalar.copy(out=ft_sb, in_=ps_ft)
        ps_out = ps_ux.tile([1, 2], F32, tag="ux")
        nc.tensor.matmul(ps_out, ft_sb, lswm_sb, start=True, stop=False)
        nc.tensor.matmul(ps_out, ones_sb[0:1, 0:1], lsb_sb,
                         start=False, stop=True)
        out_sb = scr.tile([1, 2], F32, tag="e8")
        lr_sb = scr.tile([1, 2], F32, tag="e9")
        nc.scalar.mul(out=lr_sb, in_=ps_out, mul=0.01)
        nc.vector.tensor_max(out_sb, lr_sb, ps_out)
        nc.sync.dma_start(out=out_d[:, :], in_=out_sb)

    if split_multiwaits:
        _split_matmul_multiwaits(nc)
    return nc


def _split_matmul_multiwaits(nc):
    """Walrus TPB codegen supports a single sync-wait slot per instruction; hoist extra
    waits onto standalone PE EventSemaphore instructions in front."""
    n = [0]
    for fn in nc.m.functions:
        for blk in fn.blocks:
            out = []
            for ins in blk.instructions:
                si = ins.sync_info
                if (ins.opcode not in ("EventSemaphore",)
                        and si is not None
                        and si.on_wait and len(si.on_wait) > 1):
                    waits = list(si.on_wait)
                    for w in waits[:-1]:
                        n[0] += 1
                        ev = mybir.InstEventSemaphore(
                            name=f"EVW-{n[0]}",
                            engine=ins.engine,
                            ins=[],
                            outs=[],
                            sync_info=mybir.SyncInfo(on_wait=[w], on_update=[]),
                            bass_nofuse=True,
                        )
                        out.append(ev)
                    ins.sync_info = mybir.SyncInfo(
                        on_wait=[waits[-1]], on_update=list(si.on_update or []))
                out.append(ins)
            blk.instructions[:] = out


def make_core_inputs(inputs, g):
    """Host-side prep of per-core arrays for core g (stocks 2g, 2g+1)."""
    f32 = np.float32
    sl = slice(2 * g, 2 * g + 2)
    x = np.asarray(inputs["text_input"][sl], f32)          # (2, D, T, E)
    ts = np.asarray(inputs["time_inputs"][sl], f32)        # (2, D, T)
    xt = np.ascontiguousarray(x.transpose(0, 3, 2, 1)).reshape(S_LOC, KT, 128, DT)

    uwt = np.asarray(inputs["Uall_w"][sl], f32).transpose(0, 2, 1)  # (2, E, 256)
    uw = np.ascontiguousarray(
        uwt.reshape(S_LOC, KT, 128, 2, 128).transpose(0, 3, 1, 2, 4))

    wall = np.zeros((S_LOC, 2, 128, 128), f32)
    ub = np.asarray(inputs["Uall_b"][sl], f32) + np.asarray(inputs["Wall_b"][sl], f32)
    wt = np.asarray(inputs["Wall_w"][sl], f32).transpose(0, 2, 1)   # (2, 64, 256)
    for m in range(2):
        wall[:, m, 0:64, :] = wt[:, :, m * 128:(m + 1) * 128]
        wall[:, m, 64, :] = ub[:, m * 128:(m + 1) * 128]

    wdb = np.zeros((S_LOC, 65, 64), f32)
    wdb[:, 0:64, :] = np.asarray(inputs["Wd_w"][sl], f32).transpose(0, 2, 1)
    wdb[:, 64, :] = np.asarray(inputs["Wd_b"][sl], f32)

    tmm = (ts - 1.0).transpose(2, 0, 1).reshape(1, T * 128)  # (1, t*128+s*64+d)
    tm1 = np.ascontiguousarray(np.broadcast_to(tmm, (64, T * 128)))

    lw = np.asarray(inputs["lWih"][sl], f32).transpose(0, 2, 1)     # (2, 64, 256)
    lb = np.asarray(inputs["lbih"][sl], f32) + np.asarray(inputs["lbhh"][sl], f32)
    lwihg = np.zeros((S_LOC, 3, 65, 64), f32)
    for gi, gs in enumerate([0, 2, 3]):  # i, g, o  (PyTorch order i,f,g,o)
        lwihg[:, gi, 0:64, :] = lw[:, :, gs * 64:(gs + 1) * 64]
        lwihg[:, gi, 64, :] = lb[:, gs * 64:(gs + 1) * 64]

    lsw = np.asarray(inputs["ls_w"], f32).reshape(64)
    lswm = np.zeros((128, 2), f32)
    lswm[0:64, 0] = lsw
    lswm[64:128, 1] = lsw
    lsb = np.full((1, 2), float(np.asarray(inputs["ls_b"]).reshape(-1)[0]), f32)

    eye64 = np.eye(64, dtype=f32)
    return {
        "xt": xt, "uw": uw, "wallT": wall, "wdb": wdb, "tm1": tm1,
        "ident": np.eye(128, dtype=f32),
        "vi": np.concatenate([eye64, eye64], axis=0),
        "lwihg": lwihg, "lswm": lswm, "lsb": lsb,
    }


_prog_cache = {}


def get_program(split_multiwaits=True):
    key = ("nc", split_multiwaits)
    if key not in _prog_cache:
        _prog_cache[key] = build_program(split_multiwaits)
    return _prog_cache[key]


def kernel(**inputs):
    nc = get_program()
    in_maps = [make_core_inputs(inputs, g) for g in range(N_CORES)]
    res = run_bass_kernel_spmd(nc, in_maps, list(range(N_CORES)))
    outs = [np.asarray(r["out"]).reshape(S_LOC) for r in res.results]
    return np.concatenate(outs).reshape(S, 1).astype(np.float32)


if __name__ == "__main__":
    nc = build_program()
    print("program built ok")
